# revision 16
# baseline (speedup 1.0000x reference)
# Trainium2 Bass kernel for AugmentedLSTMTagger (char-LSTM + word-LSTM + tagger).
#
# Strategy: data-parallel over the 32 sentences (4 per core, 8 cores).
# Per core (words ordered s-major, w = s*4 + b):
#   Phase A (char): one-hot matmul replaces the char-embedding gather
#     (PRE = char_emb @ Wc_ih.T computed on device, [128, 1024]); char LSTM
#     runs in transposed layout [feature, words] so all elementwise ops use
#     full 128-partition tiles. Internally h is stored as h/2 ("h-tilde") so
#     h = sigmoid(o)*(sigmoid(2c)-0.5) is one fused op; the 2x is folded into
#     the recurrent weights and downstream consumers. All transcendentals are
#     sigmoid (tanh(x) = 2*sigmoid(2x)-1, folded into weight/bias prescaling).
#   Phase A4: word input projection x=[we; cf] @ Ww_ih.T via x-stationary
#     matmuls, spilled to DRAM, streamed back during the recurrence.
#   Phase B (word): 1024 sequential steps. Gates computed via column-packed
#     matmuls (tile_position): stationary = h-tilde^T replicated across the 4
#     column groups, 4 concurrent 512-wide weight streams per K-round, plus a
#     5th "identity" round that adds the precomputed input gates from SBUF.
#     Cell update uses mixed SBUF/PSUM operands to satisfy the base-partition
#     rule. h is transposed back each step via PE transpose with a replicated
#     identity, feeding both the next stationary and the tag-phase buffer.
#   Phase C (tag): logits = 2*h-tilde @ Wt.T + bt, log-softmax per token.
import numpy as np

import concourse.bass as bass
import concourse.mybir as mybir
import concourse.tile as tile
from concourse import bacc
from concourse.bass import ds
from concourse.bass_utils import run_bass_kernel_spmd

B, S, C, E, H, V, CS, T = 32, 1024, 12, 256, 512, 50000, 128, 50
NCORES = 8
BC = B // NCORES                 # sentences per core
f32 = mybir.dt.float32
bf16 = mybir.dt.bfloat16
i32 = mybir.dt.int32
SIG = mybir.ActivationFunctionType.Sigmoid
EXP = mybir.ActivationFunctionType.Exp
LN = mybir.ActivationFunctionType.Ln
MULT = mybir.AluOpType.mult
ADD = mybir.AluOpType.add
SUB = mybir.AluOpType.subtract
MAX = mybir.AluOpType.max
IS_EQ = mybir.AluOpType.is_equal

_cache = {}


def _prep_gates(Wf, bf_, hd):
    """torch gate order i,f,g,o -> [i,f,o,2*g] rows (all-sigmoid trick)."""
    i, f, g, o = Wf[0:hd], Wf[hd:2*hd], Wf[2*hd:3*hd], Wf[3*hd:4*hd]
    bi, bff, bg, bo = bf_[0:hd], bf_[hd:2*hd], bf_[2*hd:3*hd], bf_[3*hd:4*hd]
    Wp = np.concatenate([i, f, o, 2*g], 0)
    bp = np.concatenate([bi, bff, bo, 2*bg], 0)
    return Wp.astype(np.float32), bp.astype(np.float32)


def build_nc(S_RUN=S):
    WQ = S_RUN * BC              # words per core in play
    NCH = max(1, WQ // 512)      # char n-chunks of 512 words
    NWT = WQ // 128              # 128-word tiles
    nc = bacc.Bacc(None)

    # ---- external I/O ----
    ciT = nc.dram_tensor("ciT", [C * NCH, 512], i32, kind="ExternalInput")
    wi = nc.dram_tensor("wi", [128, NWT], i32, kind="ExternalInput")
    cemb = nc.dram_tensor("cemb", [CS, E], f32, kind="ExternalInput")
    WcihT = nc.dram_tensor("WcihT", [E, 4*E], f32, kind="ExternalInput")
    WchhT = nc.dram_tensor("WchhT", [E, 4*E], f32, kind="ExternalInput")
    bc_in = nc.dram_tensor("bc", [128, 8], f32, kind="ExternalInput")
    WwihT = nc.dram_tensor("WwihT", [2*E, 4*H], f32, kind="ExternalInput")
    bw_in = nc.dram_tensor("bw", [1, 4*H], bf16, kind="ExternalInput")
    WhhT = nc.dram_tensor("WhhT", [H, 4*H], f32, kind="ExternalInput")
    wemb = nc.dram_tensor("wemb", [V, E], f32, kind="ExternalInput")
    Wt2T = nc.dram_tensor("Wt2T", [H, T], bf16, kind="ExternalInput")
    bt_in = nc.dram_tensor("bt", [1, T], bf16, kind="ExternalInput")
    out = nc.dram_tensor("out", [WQ, T], f32, kind="ExternalOutput")

    with tile.TileContext(nc) as tc:
        _pools = []

        def _pool(**kw):
            p = tc.alloc_tile_pool(**kw)
            _pools.append(p)
            return p

        cst = _pool(name="cst", bufs=1)
        dram = _pool(name="dram", bufs=1, space="DRAM")
        # PSUM pools (8 banks total)
        cgps = _pool(name="cgps", bufs=2, space="PSUM")   # 2 banks
        a4ps = _pool(name="a4ps", bufs=1, space="PSUM")   # 2 banks
        wps = _pool(name="wps", bufs=1, space="PSUM")     # 1 bank
        scrps = _pool(name="scrps", bufs=1, space="PSUM") # 1 bank
        tpps = _pool(name="tpps", bufs=1, space="PSUM")   # 2 banks (2 tags)
        # SBUF pools
        sb3 = _pool(name="sb3", bufs=3)
        ca = _pool(name="ca", bufs=1)
        cstt = _pool(name="cstt", bufs=2)
        wsmall = _pool(name="wsmall", bufs=2)
        wgp = _pool(name="wgp", bufs=2)
        wgbp = _pool(name="wgbp", bufs=2)
        hwmp = _pool(name="hwmp", bufs=2)
        hcfp = _pool(name="hcfp", bufs=2)

        wg_dram = dram.tile([WQ, 4*H], f32)
        hw_dram = dram.tile([NWT, 128, 4, 128], bf16)
        hc_dram = dram.tile([NCH, 128, 2, 512], f32)

        # ---------- constants ----------
        WchhT_sb = cst.tile([128, 2, 4*E], f32)   # [k, gates]
        nc.gpsimd.dma_start(WchhT_sb[:], WchhT[:].rearrange("(k p) g -> p k g", k=2))
        WwihT_sb = cst.tile([128, 4, 4*H], f32)
        nc.gpsimd.dma_start(WwihT_sb[:], WwihT[:].rearrange("(k p) g -> p k g", k=4))
        WhhT_sb = cst.tile([128, 4, 4, 512], f32)  # [k, j(gate), n]
        nc.gpsimd.dma_start(WhhT_sb[:], WhhT[:].rearrange("(k p) (j n) -> p k j n", k=4, j=4))
        Wt2T_sb = cst.tile([128, 4, T], bf16)
        nc.gpsimd.dma_start(Wt2T_sb[:], Wt2T[:].rearrange("(k p) t -> p k t", k=4))
        bc_sb = cst.tile([128, 8], f32)
        nc.gpsimd.dma_start(bc_sb[:], bc_in[:])
        bw_sb = cst.tile([1, 4*H], bf16)
        nc.gpsimd.dma_start(bw_sb[:], bw_in[:])
        bt_sb = cst.tile([1, T], bf16)
        nc.gpsimd.dma_start(bt_sb[:], bt_in[:])
        wi_sb = cst.tile([128, NWT], i32)
        nc.gpsimd.dma_start(wi_sb[:], wi[:])
        cemb_sb = cst.tile([CS, E], f32)
        nc.gpsimd.dma_start(cemb_sb[:], cemb[:])

        ones_bf = cst.tile([1, 128], bf16)
        nc.vector.memset(ones_bf[:], 1.0)
        iota_pi = cst.tile([128, 1], i32)
        nc.gpsimd.iota(iota_pi[:], pattern=[[1, 1]], base=0, channel_multiplier=1)
        ident128 = cst.tile([128, 128], f32)
        nc.gpsimd.memset(ident128[:], 0.0)
        nc.gpsimd.affine_select(out=ident128[:], in_=ident128[:],
                                compare_op=mybir.AluOpType.not_equal, fill=1.0,
                                base=0, pattern=[[-1, 128]], channel_multiplier=1)
        # identity [4,16] replicated: idrep[b, 4j+b] = 1
        idrep = cst.tile([4, 16], f32)
        nc.gpsimd.memset(idrep[:], 0.0)
        for j in range(4):
            nc.gpsimd.affine_select(out=idrep[:, j*4:(j+1)*4], in_=idrep[:, j*4:(j+1)*4],
                                    compare_op=mybir.AluOpType.not_equal, fill=1.0,
                                    base=0, pattern=[[-1, 4]], channel_multiplier=1)
        # selector for the wordgate identity round: isel[b, 32j+b] = 1
        isel = cst.tile([4, 128], f32)
        nc.gpsimd.memset(isel[:], 0.0)
        for j in range(4):
            nc.gpsimd.affine_select(out=isel[:, j*32:j*32+4], in_=isel[:, j*32:j*32+4],
                                    compare_op=mybir.AluOpType.not_equal, fill=1.0,
                                    base=0, pattern=[[-1, 4]], channel_multiplier=1)

        # persistent state
        stat = cst.tile([128, 4, 128], f32)       # word stationary (h-tilde^T repl.)
        nc.vector.memset(stat[:], 0.0)
        c_sb = cst.tile([128, 512], f32)          # word cell state (rows 32:36 live)
        nc.vector.memset(c_sb[:], 0.0)

        # ---------- PRE = cemb @ Wc_ih.T  ([CS, 4E]) ----------
        cembT_ps = a4ps.tile([128, 2, 128], f32, tag="a4")
        for e in range(2):
            nc.tensor.transpose(out=cembT_ps[:, e, :], in_=cemb_sb[:, e*128:(e+1)*128],
                                identity=ident128[:])
        cembT = cst.tile([128, 2, 128], f32)
        nc.vector.tensor_copy(cembT[:], cembT_ps[:])
        WcihT_sb = cst.tile([128, 2, 4*E], f32)
        nc.gpsimd.dma_start(WcihT_sb[:], WcihT[:].rearrange("(k p) g -> p k g", k=2))
        PRE = cst.tile([CS, 4*E], f32)
        for half in range(2):
            pre_ps = a4ps.tile([128, 512], f32, tag="a4")
            for k in range(2):
                nc.tensor.matmul(pre_ps[:], cembT[:, k, :],
                                 WcihT_sb[:, k, half*512:(half+1)*512],
                                 start=(k == 0), stop=(k == 1))
            nc.vector.tensor_copy(PRE[:, half*512:(half+1)*512], pre_ps[:])

        # ---------- phase A: char LSTM (For_i over n-chunks) ----------
        with tc.For_i(0, NCH, 1) as nch:
            hc = cstt.tile([128, 2, 512], f32, tag="hc")
            cc = cstt.tile([128, 2, 512], f32, tag="cc")
            nc.vector.memset(hc[:], 0.0)
            nc.vector.memset(cc[:], 0.0)
            for t in range(C):
                cib = wsmall.tile([128, 512], i32, tag="cib")
                nc.gpsimd.dma_start(
                    cib[:], ciT[:][ds(nch * C + t, 1), :].to_broadcast([128, 512]))
                oh = wsmall.tile([128, 512], f32, tag="oh")
                nc.vector.tensor_tensor(out=oh[:], in0=cib[:],
                                        in1=iota_pi[:].to_broadcast([128, 512]),
                                        op=IS_EQ)
                hc2 = cstt.tile([128, 2, 512], f32, tag="hc")
                cc2 = cstt.tile([128, 2, 512], f32, tag="cc")
                if t == C - 1:
                    hcf = hcfp.tile([128, 2, 512], f32, tag="hcf")
                else:
                    hcf = None
                A = []
                for gt in range(8):
                    gp = cgps.tile([128, 512], f32, tag="cg")
                    nc.tensor.matmul(gp[:], PRE[:, gt*128:(gt+1)*128], oh[:],
                                     start=True, stop=False)
                    for k in range(2):
                        nc.tensor.matmul(gp[:], WchhT_sb[:, k, gt*128:(gt+1)*128],
                                         hc[:, k, :], start=False, stop=(k == 1))
                    a = ca.tile([128, 512], f32, tag=f"a{gt}")
                    nc.scalar.activation(out=a[:], in_=gp[:], func=SIG,
                                         bias=bc_sb[:, gt:gt+1])
                    A.append(a)
                for e in range(2):
                    ai, af, ao, ag = A[e], A[2+e], A[4+e], A[6+e]
                    tt = ca.tile([128, 512], f32, tag="tt")
                    nc.vector.scalar_tensor_tensor(out=tt[:], in0=ag[:], scalar=0.5,
                                                   in1=ai[:], op0=SUB, op1=MULT)
                    m2 = ca.tile([128, 512], f32, tag="m2")
                    nc.vector.tensor_tensor(out=m2[:], in0=af[:], in1=cc[:, e, :], op=MULT)
                    nc.vector.scalar_tensor_tensor(out=cc2[:, e, :], in0=tt[:], scalar=2.0,
                                                   in1=m2[:], op0=MULT, op1=ADD)
                    s2 = ca.tile([128, 512], f32, tag="s2")
                    nc.scalar.activation(out=s2[:], in_=cc2[:, e, :], func=SIG, scale=2.0)
                    dst = hcf[:, e, :] if t == C - 1 else hc2[:, e, :]
                    nc.vector.scalar_tensor_tensor(out=dst, in0=s2[:], scalar=0.5,
                                                   in1=ao[:], op0=SUB, op1=MULT)
                if t == C - 1:
                    nc.gpsimd.dma_start(hc_dram[ds(nch, 1), :, :, :], hcf[:])
                hc, cc = hc2, cc2

        # ---------- phase A4: word input projection -> wg_dram ----------
        for m in range(NWT):
            we = sb3.tile([128, E], f32, tag="we")
            nc.gpsimd.indirect_dma_start(
                out=we[:], out_offset=None, in_=wemb[:],
                in_offset=bass.IndirectOffsetOnAxis(ap=wi_sb[:, m:m+1], axis=0))
            weT_ps = tpps.tile([128, 128], f32, tag="weT_ps")
            weT = sb3.tile([128, 2, 128], f32, tag="weT")
            for e in range(2):
                nc.tensor.transpose(out=weT_ps[:], in_=we[:, e*128:(e+1)*128],
                                    identity=ident128[:])
                nc.vector.tensor_copy(weT[:, e, :], weT_ps[:])
            hcm = sb3.tile([128, 2, 128], f32, tag="hcm")
            nc.gpsimd.dma_start(hcm[:], hc_dram[m // 4, :, :, (m % 4)*128:(m % 4 + 1)*128])
            for half in range(2):
                ps = a4ps.tile([128, 1024], f32, tag="a4")
                for k in range(4):
                    lt = weT[:, k, :] if k < 2 else hcm[:, k-2, :]
                    for n in range(2):
                        nc.tensor.matmul(
                            ps[:, n*512:(n+1)*512], lt,
                            WwihT_sb[:, k, half*1024 + n*512: half*1024 + (n+1)*512],
                            start=(k == 0), stop=False)
                for n in range(2):
                    nc.tensor.matmul(
                        ps[:, n*512:(n+1)*512], ones_bf[:],
                        bw_sb[:, half*1024 + n*512: half*1024 + (n+1)*512],
                        start=False, stop=True)
                wgh = wgbp.tile([128, 1024], f32, tag="wgbig")
                if half == 0:
                    nc.vector.tensor_copy(wgh[:], ps[:])
                else:
                    nc.scalar.copy(wgh[:], ps[:])
                nc.gpsimd.dma_start(wg_dram[m*128:(m+1)*128, half*1024:(half+1)*1024],
                                    wgh[:])

        # ---------- phase B: word recurrence ----------
        with tc.For_i(0, NWT, 1) as m:
            hwm = hwmp.tile([128, 4, 128], bf16, tag="hwm")
            for u in range(32):
                wgt = wgp.tile([4, 4*H], f32, tag="wg")
                nc.gpsimd.dma_start(wgt[:], wg_dram[ds(m * 128 + 4 * u, 4), :])
                G = wps.tile([128, 512], f32, tag="G")
                for k in range(4):
                    for j in range(4):
                        nc.tensor.matmul(G[32*j:32*j+32, :], stat[:, k, 32*j:32*j+32],
                                         WhhT_sb[:, k, j, :], start=(k == 0), stop=False,
                                         tile_position=(0, 32*j))
                for j in range(4):
                    nc.tensor.matmul(G[32*j:32*j+32, :], isel[:, 32*j:32*j+32],
                                     wgt[:, 512*j:512*(j+1)],
                                     start=False, stop=True, tile_position=(0, 32*j))
                A_sb = wsmall.tile([128, 512], f32, tag="A")
                nc.scalar.activation(out=A_sb[0:68, :], in_=G[0:68, :], func=SIG)
                scr = scrps.tile([128, 512], f32, tag="scr")
                nc.scalar.activation(out=scr[0:4, :], in_=G[96:100, :], func=SIG)
                tt = wsmall.tile([4, 512], f32, tag="t")
                nc.vector.scalar_tensor_tensor(out=tt[:], in0=scr[0:4, :], scalar=0.5,
                                               in1=A_sb[0:4, :], op0=SUB, op1=MULT)
                nc.vector.tensor_tensor(out=scr[32:36, :], in0=A_sb[32:36, :],
                                        in1=c_sb[32:36, :], op=MULT)
                nc.vector.scalar_tensor_tensor(out=c_sb[32:36, :], in0=tt[:], scalar=2.0,
                                               in1=scr[32:36, :], op0=MULT, op1=ADD)
                nc.scalar.activation(out=scr[64:68, :], in_=c_sb[32:36, :], func=SIG,
                                     scale=2.0)
                ht = wsmall.tile([4, 512], f32, tag="ht")
                nc.vector.scalar_tensor_tensor(out=ht[:], in0=scr[64:68, :], scalar=0.5,
                                               in1=A_sb[64:68, :], op0=SUB, op1=MULT)
                tp = tpps.tile([128, 64], f32, tag="tp")
                for k in range(4):
                    nc.tensor.transpose(out=tp[:, 16*k:16*(k+1)], in_=ht[:, 128*k:128*(k+1)],
                                        identity=idrep[:])
                nc.vector.tensor_copy(
                    out=stat[:].rearrange("p k (j r) -> p k j r", j=4)[:, :, :, 0:4],
                    in_=tp[:].rearrange("p (k j b) -> p k j b", k=4, j=4))
                nc.vector.tensor_copy(
                    out=hwm[:, :, 4*u:4*u+4],
                    in_=tp[:].rearrange("p (k j b) -> p k j b", k=4, j=4)[:, :, 0, :])
            nc.gpsimd.dma_start(hw_dram[ds(m, 1), :, :, :], hwm[:])

        # ---------- phase C: tag projection + log_softmax ----------
        for m in range(NWT):
            hwm2 = hwmp.tile([128, 4, 128], bf16, tag="hwm2")
            nc.gpsimd.dma_start(hwm2[:], hw_dram[m, :, :, :])
            lg = wps.tile([128, T], f32, tag="G")
            for k in range(4):
                nc.tensor.matmul(lg[:], hwm2[:, k, :], Wt2T_sb[:, k, :],
                                 start=(k == 0), stop=False)
            nc.tensor.matmul(lg[:], ones_bf[:], bt_sb[:], start=False, stop=True)
            mx = wsmall.tile([128, 1], f32, tag="mx")
            nc.vector.tensor_reduce(out=mx[:], in_=lg[:], axis=mybir.AxisListType.X,
                                    op=MAX)
            nmx = wsmall.tile([128, 1], f32, tag="nmx")
            nc.vector.tensor_scalar(out=nmx[:], in0=mx[:], scalar1=-1.0, scalar2=None,
                                    op0=MULT)
            ex = wsmall.tile([128, T], f32, tag="ex")
            nc.scalar.activation(out=ex[:], in_=lg[:], func=EXP, bias=nmx[:, 0:1])
            sm = wsmall.tile([128, 1], f32, tag="sm")
            nc.vector.tensor_reduce(out=sm[:], in_=ex[:], axis=mybir.AxisListType.X,
                                    op=ADD)
            lns = wsmall.tile([128, 1], f32, tag="lns")
            nc.scalar.activation(out=lns[:], in_=sm[:], func=LN)
            ot = wsmall.tile([128, T], f32, tag="ot")
            nc.vector.tensor_scalar(out=ot[:], in0=lg[:], scalar1=nmx[:, 0:1],
                                    scalar2=lns[:, 0:1], op0=ADD, op1=SUB)
            nc.gpsimd.dma_start(out[m*128:(m+1)*128, :], ot[:])

        for _p in reversed(_pools):
            _p.release()

    nc.finalize()
    return nc


def ml_bf16():
    import ml_dtypes
    return ml_dtypes.bfloat16


def _prep_inputs(inputs, S_RUN=S):
    WQ = S_RUN * BC
    NCH = max(1, WQ // 512)
    NWT = WQ // 128
    ci = np.asarray(inputs["char_idx"], np.int32)
    wi = np.asarray(inputs["word_idx"], np.int32)
    Wc, bc = _prep_gates(np.asarray(inputs["Wc_ih"], np.float32),
                         np.asarray(inputs["bc"], np.float32), E)
    Uc, _ = _prep_gates(np.asarray(inputs["Wc_hh"], np.float32),
                        np.zeros(4*E, np.float32), E)
    Uc = 2.0 * Uc
    Ww, bw = _prep_gates(np.asarray(inputs["Ww_ih"], np.float32),
                         np.asarray(inputs["bw"], np.float32), H)
    Ww = Ww.copy()
    Ww[:, E:2*E] *= 2.0
    Uw, _ = _prep_gates(np.asarray(inputs["Ww_hh"], np.float32),
                        np.zeros(4*H, np.float32), H)
    Uw = 2.0 * Uw
    shared = {
        "cemb": np.asarray(inputs["char_emb"], np.float32),
        "WcihT": np.ascontiguousarray(Wc.T),
        "WchhT": np.ascontiguousarray(Uc.T),
        "bc": np.ascontiguousarray(bc.reshape(8, 128).T),
        "WwihT": np.ascontiguousarray(Ww.T),
        "bw": bw.reshape(1, 4*H).astype(ml_bf16()),
        "WhhT": np.ascontiguousarray(Uw.T),
        "wemb": np.asarray(inputs["word_emb"], np.float32),
        "Wt2T": np.ascontiguousarray((2.0 * np.asarray(inputs["Wt"], np.float32)).T
                                     ).astype(ml_bf16()),
        "bt": np.asarray(inputs["bt"], np.float32).reshape(1, T).astype(ml_bf16()),
    }
    maps = []
    for core in range(NCORES):
        cic = ci[BC*core:BC*core+BC, :S_RUN]            # [4, S_RUN, 12]
        wic = wi[BC*core:BC*core+BC, :S_RUN]            # [4, S_RUN]
        ci_w = cic.transpose(1, 0, 2).reshape(WQ, C)    # w-major, [WQ, 12]
        ciT = np.zeros((C * NCH, 512), np.int32)
        for nch in range(NCH):
            ciT[nch*C:(nch+1)*C, :] = ci_w[nch*512:(nch+1)*512, :].T
        wi_w = wic.transpose(1, 0).reshape(WQ)
        wim = np.ascontiguousarray(wi_w.reshape(NWT, 128).T).astype(np.int32)
        maps.append({"ciT": ciT, "wi": wim, **shared})
    return maps


LAST_RESULTS = None


def kernel(**inputs) -> np.ndarray:
    global LAST_RESULTS
    import os
    S_RUN = int(os.environ.get("KERNEL_S_RUN", S))
    if S_RUN not in _cache:
        _cache[S_RUN] = build_nc(S_RUN)
    nc = _cache[S_RUN]
    maps = _prep_inputs(inputs, S_RUN)
    kw = {}
    if os.environ.get("KERNEL_TRACE") == "1":
        kw["trace"] = True
        if os.environ.get("KERNEL_TRACE_DIR"):
            kw["tmpdir"] = os.environ["KERNEL_TRACE_DIR"]
    res = run_bass_kernel_spmd(nc, maps, core_ids=list(range(NCORES)), **kw)
    LAST_RESULTS = res
    outs = []
    for core in range(NCORES):
        o = np.asarray(res.results[core]["out"])        # [WQ, T]
        outs.append(o.reshape(S_RUN, BC, T).transpose(1, 0, 2))
    return np.concatenate(outs, 0).astype(np.float32)   # [B, S_RUN, T]
